# revision 16
# baseline (speedup 1.0000x reference)
"""Causal self-attention on 8 Trainium2 NeuronCores — zero-collective design.

Problem: x [4, 2048, 1024] fp32; Wq/Wk/Wv [1024, 1024].
  q,k,v = x@W*; S = q@k^T; causal mask; attn = softmax(S/32); out = attn@v.

Key algebra (removes all inter-core communication):
  S   = (x_q Wq)(x_k Wk)^T = x_q M x_k^T      with M = Wq Wk^T (host, fp32)
  out = softmax(S/32) x_k Wv = G Wv           with G = P x_k   (P = masked exp)
so the kernel never materializes Q, K or V. Per-core work:
  R^T = M^T x_q^T   (128 MMs)   -- x_q^T columns are this core's 1024 q rows
  per step: S^T = x_k^T-stationary @ R^T; P^T = exp(S^T/32)*mask;
            G^T += x_v-stationary @ P^T; l += ones^T P_acc
  out = (G Wv) / l  (128 MMs, scaled on ACT during PSUM evacuation)

Sharding: 2 cores per batch element, wedge query pairing (chunks (0,3) /
(1,2)) so both cores see 5 live kv-block visits padded to a uniform
6-step schedule. The kv operands are host-provided INPUT slices (x^T
d-major for S, x row-major for G), so there is nothing to gather.
Steps pair as (ones,ones) / (diag,dead) across the two core types;
the two (diag,dead) steps (si=1,5) compute only the lower-triangle
kv-tile ranges (free-dim narrowing saves ~10us of PE).

Everything on the matmul data plane is bf16 (measured end-to-end rel
err ~4e-3); accumulation (PSUM, G, l) is fp32.
"""

import numpy as np

B, N, D = 4, 2048, 1024
P = 128
CHUNK = 512
NCORES = 8

# step si -> (kv_block, chunk_slot); slot 0 = lo chunk cols, 1 = hi
STEP_DEF = [(0, 0), (1, 0), (0, 1), (1, 1), (2, 1), (3, 1)]
FIRST_OF_CHUNK = {0: 0, 1: 2}   # chunk -> first si
LAST_OF_CHUNK = {0: 1, 1: 5}    # chunk -> last si
MASKED_STEPS = {0: 0, 1: 1, 4: 2, 5: 3}  # si -> mask index (2,3 are all-ones)
NARROW_STEPS = {1, 5}  # (diag, dead) pairs: ks only needs q >= ks*128

_CACHE = {}


def _build_program():
    import concourse.bacc as bacc
    import concourse.mybir as mybir
    import concourse.tile as tile

    F32 = mybir.dt.float32
    BF16 = mybir.dt.bfloat16
    EXP = mybir.ActivationFunctionType.Exp
    COPY = mybir.ActivationFunctionType.Copy

    nc = bacc.Bacc("TRN2", target_bir_lowering=False, debug=False,
                   num_devices=NCORES)

    # d-major x^T columns of this core's q rows (lo 512 | hi 512)
    xqt = nc.declare_dram_parameter("xqt", [D, 1024], BF16, isOutput=False)
    # d-major x^T of the full batch element, natural row order
    xkt = nc.declare_dram_parameter("xkt", [D, N], BF16, isOutput=False)
    # row-major x of the full batch element
    xv = nc.declare_dram_parameter("xv", [N, D], BF16, isOutput=False)
    m_in = nc.declare_dram_parameter("m_in", [D, D], BF16, isOutput=False)
    wv = nc.declare_dram_parameter("wv", [D, D], BF16, isOutput=False)
    masks = nc.declare_dram_parameter("masks", [4, P, 4, CHUNK], BF16,
                                      isOutput=False)
    out = nc.declare_dram_parameter("out", [1024, D], F32, isOutput=True)

    xqt_r = xqt.rearrange("(ds p) n -> p ds n", p=P)   # [128, 8, 1024]
    xkt_r = xkt.rearrange("(ds p) n -> p ds n", p=P)   # [128, 8, 2048]
    xv_r = xv.rearrange("(t p) d -> p t d", p=P)       # [128, 16, 1024]
    m_r = m_in.rearrange("(ds p) e -> p ds e", p=P)    # [128, 8, 1024]
    wv_r = wv.rearrange("(ds p) e -> p ds e", p=P)     # [128, 8, 1024]

    with tile.TileContext(nc) as tc:
        with (
            tc.tile_pool(name="persist", bufs=1) as persist,
            tc.tile_pool(name="xq", bufs=16) as xq_pool,
            tc.tile_pool(name="xvp", bufs=4) as xv_pool,
            tc.tile_pool(name="mp", bufs=2) as m_pool,
            tc.tile_pool(name="ptp", bufs=8) as pt_pool,
            tc.tile_pool(name="pacc", bufs=2) as pacc_pool,
            tc.tile_pool(name="oout", bufs=3) as o_pool,
            tc.tile_pool(name="small", bufs=1) as small_pool,
            tc.tile_pool(name="mm", bufs=4, space="PSUM") as psum_mm,
            tc.tile_pool(name="pg", bufs=2, space="PSUM") as psum_g,
            tc.tile_pool(name="pl", bufs=2, space="PSUM") as psum_l,
            tc.tile_pool(name="dram", bufs=1, space="DRAM") as dram_pool,
        ):
            xkt_sb = persist.tile([P, 8, N], BF16)
            m_sb = persist.tile([P, 8, D], BF16)
            rt_sb = persist.tile([P, 8, 1024], BF16)
            gt_sb = persist.tile([P, 8, 1024], F32)
            gt16_sb = persist.tile([P, 8, 1024], BF16)
            wv_sb = persist.tile([P, 8, D], BF16)
            ones_f32 = persist.tile([P, 1], F32)
            nc.vector.memset(ones_f32[:], 1.0)
            ones_sb = persist.tile([P, 1], BF16)
            nc.vector.tensor_copy(out=ones_sb[:], in_=ones_f32[:])

            # PE warm-up: throwaway matmuls during the input-DMA wait flip
            # the HAM clock gate to 8/8 before real work arrives.
            warm_sb = persist.tile([P, CHUNK], BF16, name="warm_sb")
            nc.vector.memset(warm_sb[:], 0.0)
            warm_ps = psum_mm.tile([P, CHUNK], F32, tag="mm", name="warm_ps")
            for i in range(17):
                nc.tensor.matmul(warm_ps[:], warm_sb[:, 0:P], warm_sb[:],
                                 start=True, stop=True)
            warm_out = persist.tile([P, 1], F32, name="warm_out")
            nc.vector.tensor_copy(out=warm_out[:], in_=warm_ps[:, 0:1])

            # ---- input DMAs: ONE queue, strict priority order. Splitting
            # across queues fair-shares DMA bandwidth and starves the
            # R-critical loads (measured: 32us PE idle). FIFO on the sync
            # queue delivers first-needed bytes at full rate.
            def load_m_slice(es):
                nc.sync.dma_start(m_sb[:, :, es * P:(es + 1) * P],
                                  m_r[:, :, es * P:(es + 1) * P])

            def load_xq(c):
                # per-di tiles so the first R matmuls can start while the
                # rest of the chunk is still in flight
                ts = []
                for di in range(8):
                    xt = xq_pool.tile([P, CHUNK], BF16, tag="xq",
                                      name=f"xq_{c}_{di}")
                    nc.sync.dma_start(
                        xt[:], xqt_r[:, di, c * CHUNK:(c + 1) * CHUNK])
                    ts.append(xt)
                return ts

            def load_xkt(b):
                nc.sync.dma_start(
                    xkt_sb[:, :, b * CHUNK:(b + 1) * CHUNK],
                    xkt_r[:, :, b * CHUNK:(b + 1) * CHUNK])

            def load_xv(b):
                xt = xv_pool.tile([P, 4, D], BF16, tag="xv", name=f"xv_{b}")
                nc.sync.dma_start(xt[:], xv_r[:, 4 * b:4 * b + 4, :])
                return xt

            def load_mask(mi):
                mt = m_pool.tile([P, 4, CHUNK], BF16, tag="m", name=f"m_{mi}")
                nc.sync.dma_start(mt[:], masks[mi])
                return mt

            load_m_slice(0)
            xq_t = [load_xq(0)]
            for es in range(1, 8):
                load_m_slice(es)
            xq_t.append(load_xq(1))
            # attention operands, in step consumption order
            load_xkt(0)
            xv_t = {0: load_xv(0)}
            mask_t = {0: load_mask(0), 1: load_mask(1)}
            load_xkt(1)
            xv_t[1] = load_xv(1)
            load_xkt(2)
            load_xkt(3)
            xv_t[2] = load_xv(2)
            xv_t[3] = load_xv(3)
            mask_t[2] = load_mask(2)
            mask_t[3] = load_mask(3)
            for ds in range(8):
                nc.sync.dma_start(wv_sb[:, ds, :], wv_r[:, ds, :])

            # ---- R^T = M^T xq^T :  [d' , q]  (contraction over d_in) ----
            for c in range(2):
                for dso in range(8):
                    ps = psum_mm.tile([P, CHUNK], F32, tag="mm",
                                      name=f"psr_{c}_{dso}")
                    for di in range(8):
                        nc.tensor.matmul(
                            ps[:], m_sb[:, di, dso * P:(dso + 1) * P],
                            xq_t[c][di][:],
                            start=(di == 0), stop=(di == 7))
                    nc.any.tensor_copy(
                        out=rt_sb[:, dso, c * CHUNK:(c + 1) * CHUNK],
                        in_=ps[:])

            # ---------------- attention steps ----------------
            l_ps = [None, None]
            for si, (b, c) in enumerate(STEP_DEF):
                first = FIRST_OF_CHUNK[c] == si
                last = LAST_OF_CHUNK[c] == si
                narrow = si in NARROW_STEPS
                xvt = xv_t[b]
                m_sbt = mask_t[MASKED_STEPS[si]] if si in MASKED_STEPS else None
                if first:
                    l_ps[c] = psum_l.tile([1, CHUNK], F32, tag="l",
                                          name=f"l{c}")
                qcol = slice(c * CHUNK, (c + 1) * CHUNK)
                # S^T tiles + exp + mask + P accumulation
                pts = []
                pacc = pacc_pool.tile([P, CHUNK], BF16, tag="pacc",
                                      name=f"pacc_{si}")
                for ks in range(4):
                    q0 = ks * P if narrow else 0
                    ps_s = psum_mm.tile([P, CHUNK], F32, tag="mm",
                                        name=f"pss_{si}_{ks}")
                    for ds in range(8):
                        nc.tensor.matmul(
                            ps_s[:, q0:],
                            xkt_sb[:, ds, (b * 4 + ks) * P:(b * 4 + ks + 1) * P],
                            rt_sb[:, ds, c * CHUNK + q0:(c + 1) * CHUNK],
                            start=(ds == 0), stop=(ds == 7))
                    pt = pt_pool.tile([P, CHUNK], BF16, tag="pt",
                                      name=f"pt_{si}_{ks}")
                    nc.scalar.activation(pt[:, q0:], ps_s[:, q0:], EXP,
                                         scale=0.03125)
                    if m_sbt is not None:
                        nc.vector.tensor_mul(
                            out=pt[:, q0:], in0=pt[:, q0:],
                            in1=m_sbt[:, ks, q0:])
                    if ks == 0:
                        pass  # pacc starts as pt0 via copy below
                    pts.append(pt)
                nc.vector.tensor_copy(out=pacc[:], in_=pts[0][:])
                for ks in range(1, 4):
                    q0 = ks * P if narrow else 0
                    nc.vector.tensor_add(out=pacc[:, q0:], in0=pacc[:, q0:],
                                         in1=pts[ks][:, q0:])
                # G^T += xv-stationary @ P^T   (per d'-tile)
                for ds in range(8):
                    ps_g = psum_g.tile([P, CHUNK], F32, tag="g",
                                       name=f"psg_{si}_{ds}")
                    for ks in range(4):
                        q0 = ks * P if narrow else 0
                        nc.tensor.matmul(
                            ps_g[:, q0:],
                            xvt[:, ks, ds * P:(ds + 1) * P],
                            pts[ks][:, q0:],
                            start=(ks == 0), stop=(ks == 3))
                    dst = gt_sb[:, ds, qcol]
                    if first:
                        nc.vector.tensor_copy(out=dst, in_=ps_g[:])
                    else:
                        nc.vector.tensor_add(out=dst, in0=dst, in1=ps_g[:])
                    if last:
                        nc.vector.tensor_copy(out=gt16_sb[:, ds, qcol],
                                              in_=dst)
                # l-MM after the G block: its pacc (DVE) dependency is
                # ready by then, so the PE queue never stalls on it.
                nc.tensor.matmul(l_ps[c][:], ones_sb[:], pacc[:],
                                 start=first, stop=last)
                # l finalize: [1,512] -> DRAM bounce -> [128,4] + reciprocal
                if last:
                    l_row = small_pool.tile([1, CHUNK], F32, tag=f"lrow{c}",
                                            name=f"lrow{c}")
                    nc.vector.tensor_copy(out=l_row[:], in_=l_ps[c][:])
                    l_b = dram_pool.tile([CHUNK], F32, tag=f"lb{c}",
                                         name=f"lb{c}")
                    nc.sync.dma_start(
                        l_b[:].rearrange("(a n) -> a n", a=1), l_row[:])
                    l_col = small_pool.tile([P, 4], F32, tag=f"lcol{c}",
                                            name=f"lcol{c}")
                    nc.sync.dma_start(
                        l_col[:], l_b[:].rearrange("(s p) -> p s", p=P))
                    linv = small_pool.tile([P, 4], F32, tag=f"linv{c}",
                                           name=f"linv{c}")
                    nc.vector.reciprocal(linv[:], l_col[:])
                    if c == 0:
                        linv0 = linv
                    else:
                        linv1 = linv

            # ---------------- out = (G Wv) / l ----------------
            for c in range(2):
                linv = linv0 if c == 0 else linv1
                for qs in range(4):
                    for eh in range(2):
                        ps_o = psum_mm.tile([P, CHUNK], F32, tag="mm",
                                            name=f"pso_{c}_{qs}_{eh}")
                        for ds in range(8):
                            nc.tensor.matmul(
                                ps_o[:],
                                gt16_sb[:, ds,
                                        c * CHUNK + qs * P:
                                        c * CHUNK + (qs + 1) * P],
                                wv_sb[:, ds, eh * CHUNK:(eh + 1) * CHUNK],
                                start=(ds == 0), stop=(ds == 7))
                        o_t = o_pool.tile([P, CHUNK], F32, tag="o",
                                          name=f"o_{c}_{qs}_{eh}")
                        nc.scalar.activation(o_t[:], ps_o[:], COPY,
                                             scale=linv[:, qs:qs + 1])
                        r0 = c * CHUNK + qs * P
                        nc.scalar.dma_start(
                            out[r0:r0 + P, eh * CHUNK:(eh + 1) * CHUNK],
                            o_t[:])

    nc.compile()
    return nc


def _get_program():
    if "nc" not in _CACHE:
        _CACHE["nc"] = _build_program()
    return _CACHE["nc"]


def _core_q_rows(core):
    b, half = divmod(core, 2)
    if half == 0:
        lo, hi = 0, 3
    else:
        lo, hi = 1, 2
    return b, lo, hi


def _build_mask(core):
    """masks [4, 128, 4, 512] bf16 for steps si in (0,1,4,5):
    m[mi, p, ks, q] = 1 iff kv_global <= q_global."""
    import ml_dtypes

    _, lo, hi = _core_q_rows(core)
    chunk_start = {0: lo * CHUNK, 1: hi * CHUNK}
    m = np.zeros((4, P, 4, CHUNK), dtype=np.float32)
    kv_local = np.arange(CHUNK)
    q_local = np.arange(CHUNK)
    for si, mi in MASKED_STEPS.items():
        b, c = STEP_DEF[si]
        kv_g = b * CHUNK + kv_local
        q_g = chunk_start[c] + q_local
        allowed = (kv_g[:, None] <= q_g[None, :]).astype(np.float32)
        m[mi] = allowed.reshape(4, P, CHUNK).transpose(1, 0, 2)
    return m.astype(ml_dtypes.bfloat16)


def _make_in_maps(x, wq, wk, wv):
    import ml_dtypes

    m_fold = (wq @ wk.T).astype(ml_dtypes.bfloat16)
    wv16 = wv.astype(ml_dtypes.bfloat16)
    in_maps = []
    for core in range(NCORES):
        b, lo, hi = _core_q_rows(core)
        xb = x[b]
        xbT = np.ascontiguousarray(xb.T).astype(ml_dtypes.bfloat16)
        xqt = np.ascontiguousarray(np.concatenate(
            [xbT[:, lo * CHUNK:(lo + 1) * CHUNK],
             xbT[:, hi * CHUNK:(hi + 1) * CHUNK]], axis=1))
        in_maps.append({
            "xqt": xqt,
            "xkt": xbT,
            "xv": np.ascontiguousarray(xb).astype(ml_dtypes.bfloat16),
            "m_in": m_fold,
            "wv": wv16,
            "masks": _build_mask(core),
        })
    return in_maps


def kernel(x, W_query, W_key, W_value):
    from concourse.bass_utils import run_bass_kernel_spmd

    x = np.ascontiguousarray(np.asarray(x, dtype=np.float32))
    wq = np.ascontiguousarray(np.asarray(W_query, dtype=np.float32))
    wk = np.ascontiguousarray(np.asarray(W_key, dtype=np.float32))
    wv = np.ascontiguousarray(np.asarray(W_value, dtype=np.float32))

    nc = _get_program()
    in_maps = _make_in_maps(x, wq, wk, wv)
    res = run_bass_kernel_spmd(nc, in_maps, core_ids=list(range(NCORES)))

    out = np.empty((B, N, D), dtype=np.float32)
    for core in range(NCORES):
        b, lo, hi = _core_q_rows(core)
        o = res.results[core]["out"]
        out[b, lo * CHUNK:(lo + 1) * CHUNK] = o[:CHUNK]
        out[b, hi * CHUNK:(hi + 1) * CHUNK] = o[CHUNK:]
    return out


# revision 22
# speedup vs baseline: 1.0158x; 1.0158x over previous
"""Causal self-attention on 8 Trainium2 NeuronCores — zero-collective design.

Problem: x [4, 2048, 1024] fp32; Wq/Wk/Wv [1024, 1024].
  q,k,v = x@W*; S = q@k^T; causal mask; attn = softmax(S/32); out = attn@v.

Key algebra (removes all inter-core communication):
  S   = (x_q Wq)(x_k Wk)^T = x_q M x_k^T      with M = Wq Wk^T (host, fp32)
  out = softmax(S/32) x_k Wv = G Wv           with G = P x_k   (P = masked exp)
so the kernel never materializes Q, K or V. Per-core work:
  R^T = M^T x_q^T   (128 MMs)   -- x_q^T columns are this core's 1024 q rows
  per step: S^T = x_k^T-stationary @ R^T; P^T = exp(S^T/32)*mask;
            G^T += x_v-stationary @ P^T; l += ones^T P_acc
  out = (G Wv) / l  (128 MMs, scaled on ACT during PSUM evacuation)

Sharding: 2 cores per batch element, wedge query pairing (chunks (0,3) /
(1,2)) so both cores see 5 live kv-block visits padded to a uniform
6-step schedule. The kv operands are host-provided INPUT slices (x^T
d-major for S, x row-major for G), so there is nothing to gather.
Steps pair as (ones,ones) / (diag,dead) across the two core types;
the two (diag,dead) steps (si=1,5) compute only the lower-triangle
kv-tile ranges (free-dim narrowing saves ~10us of PE).

Everything on the matmul data plane is bf16 (measured end-to-end rel
err ~4e-3); accumulation (PSUM, G, l) is fp32.
"""

import numpy as np

B, N, D = 4, 2048, 1024
P = 128
CHUNK = 512
NCORES = 8

# step si -> (kv_block, chunk_slot); slot 0 = lo chunk cols, 1 = hi
STEP_DEF = [(0, 0), (1, 0), (0, 1), (1, 1), (2, 1), (3, 1)]
FIRST_OF_CHUNK = {0: 0, 1: 2}   # chunk -> first si
LAST_OF_CHUNK = {0: 1, 1: 5}    # chunk -> last si
MASKED_STEPS = {0: 0, 1: 1, 4: 2, 5: 3}  # si -> mask index (2,3 are all-ones)
NARROW_STEPS = {1, 5}  # (diag, dead) pairs: ks only needs q >= ks*128

_CACHE = {}


def _build_program():
    import concourse.bacc as bacc
    import concourse.mybir as mybir
    import concourse.tile as tile

    F32 = mybir.dt.float32
    BF16 = mybir.dt.bfloat16
    EXP = mybir.ActivationFunctionType.Exp
    COPY = mybir.ActivationFunctionType.Copy

    nc = bacc.Bacc("TRN2", target_bir_lowering=False, debug=False,
                   num_devices=NCORES)

    # d-major x^T columns of this core's q rows, host-packed into 16
    # contiguous [128, 512] tiles (t = chunk*8 + di) for linear DMA
    xqt = nc.declare_dram_parameter("xqt", [16 * P, CHUNK], BF16,
                                    isOutput=False)
    # d-major x^T of the full batch element, natural row order
    xkt = nc.declare_dram_parameter("xkt", [D, N], BF16, isOutput=False)
    # row-major x of the full batch element
    xv = nc.declare_dram_parameter("xv", [N, D], BF16, isOutput=False)
    # M = Wq Wk^T, host-packed into 8 contiguous es-column blocks
    # [1024, 128] so each slice DMA is one linear 256KB read
    m_in = nc.declare_dram_parameter("m_in", [8 * D, P], BF16,
                                     isOutput=False)
    wv = nc.declare_dram_parameter("wv", [D, D], BF16, isOutput=False)
    masks = nc.declare_dram_parameter("masks", [4, P, 4, CHUNK], BF16,
                                      isOutput=False)
    out = nc.declare_dram_parameter("out", [1024, D], F32, isOutput=True)

    xqt_r = xqt.rearrange("(t p) n -> t p n", p=P)     # [16, 128, 512]
    xkt_r = xkt.rearrange("(ds p) n -> p ds n", p=P)   # [128, 8, 2048]
    xv_r = xv.rearrange("(t p) d -> p t d", p=P)       # [128, 16, 1024]
    m_r = m_in.rearrange("(es di p) e -> es p di e", di=8, p=P)  # [8,128,8,128]
    wv_r = wv.rearrange("(ds p) e -> p ds e", p=P)     # [128, 8, 1024]

    with tile.TileContext(nc) as tc:
        with (
            tc.tile_pool(name="persist", bufs=1) as persist,
            tc.tile_pool(name="xq", bufs=16) as xq_pool,
            tc.tile_pool(name="xvp", bufs=4) as xv_pool,
            tc.tile_pool(name="mp", bufs=2) as m_pool,
            tc.tile_pool(name="ptp", bufs=8) as pt_pool,
            tc.tile_pool(name="pacc", bufs=2) as pacc_pool,
            tc.tile_pool(name="oout", bufs=3) as o_pool,
            tc.tile_pool(name="small", bufs=1) as small_pool,
            tc.tile_pool(name="mm", bufs=4, space="PSUM") as psum_mm,
            tc.tile_pool(name="pg", bufs=2, space="PSUM") as psum_g,
            tc.tile_pool(name="pl", bufs=2, space="PSUM") as psum_l,
            tc.tile_pool(name="dram", bufs=1, space="DRAM") as dram_pool,
        ):
            xkt_sb = persist.tile([P, 8, N], BF16)
            m_sb = persist.tile([P, 8, D], BF16)
            rt_sb = persist.tile([P, 8, 1024], BF16)
            gt_sb = persist.tile([P, 8, 1024], F32)
            gt16_sb = persist.tile([P, 8, 1024], BF16)
            wv_sb = persist.tile([P, 8, D], BF16)
            ones_f32 = persist.tile([P, 1], F32)
            nc.vector.memset(ones_f32[:], 1.0)
            ones_sb = persist.tile([P, 1], BF16)
            nc.vector.tensor_copy(out=ones_sb[:], in_=ones_f32[:])

            # PE warm-up: throwaway matmuls during the input-DMA wait flip
            # the HAM clock gate to 8/8 before real work arrives.
            warm_sb = persist.tile([P, CHUNK], BF16, name="warm_sb")
            nc.vector.memset(warm_sb[:], 0.0)
            warm_ps = psum_mm.tile([P, CHUNK], F32, tag="mm", name="warm_ps")
            for i in range(17):
                nc.tensor.matmul(warm_ps[:], warm_sb[:, 0:P], warm_sb[:],
                                 start=True, stop=True)
            warm_out = persist.tile([P, 1], F32, name="warm_out")
            nc.vector.tensor_copy(out=warm_out[:], in_=warm_ps[:, 0:1])

            # ---- input DMAs: ONE queue, strict priority order. Splitting
            # across queues fair-shares DMA bandwidth and starves the
            # R-critical loads (measured: 32us PE idle). FIFO on the sync
            # queue delivers first-needed bytes at full rate.
            def load_m_slice(es):
                nc.sync.dma_start(m_sb[:, :, es * P:(es + 1) * P],
                                  m_r[es])

            def load_xq(c):
                # per-di tiles so the first R matmuls can start while the
                # rest of the chunk is still in flight
                ts = []
                for di in range(8):
                    xt = xq_pool.tile([P, CHUNK], BF16, tag="xq",
                                      name=f"xq_{c}_{di}")
                    nc.sync.dma_start(xt[:], xqt_r[c * 8 + di])
                    ts.append(xt)
                return ts

            def load_xkt(b):
                nc.sync.dma_start(
                    xkt_sb[:, :, b * CHUNK:(b + 1) * CHUNK],
                    xkt_r[:, :, b * CHUNK:(b + 1) * CHUNK])

            def load_xv(b):
                xt = xv_pool.tile([P, 4, D], BF16, tag="xv", name=f"xv_{b}")
                nc.sync.dma_start(xt[:], xv_r[:, 4 * b:4 * b + 4, :])
                return xt

            def load_mask(mi):
                mt = m_pool.tile([P, 4, CHUNK], BF16, tag="m", name=f"m_{mi}")
                nc.sync.dma_start(mt[:], masks[mi])
                return mt

            load_m_slice(0)
            xq_t = [load_xq(0)]
            for es in range(1, 8):
                load_m_slice(es)
            xq_t.append(load_xq(1))
            # attention operands, in step consumption order
            load_xkt(0)
            xv_t = {0: load_xv(0)}
            mask_t = {0: load_mask(0), 1: load_mask(1)}
            load_xkt(1)
            xv_t[1] = load_xv(1)
            load_xkt(2)
            load_xkt(3)
            xv_t[2] = load_xv(2)
            xv_t[3] = load_xv(3)
            mask_t[2] = load_mask(2)
            mask_t[3] = load_mask(3)
            for ds in range(8):
                nc.sync.dma_start(wv_sb[:, ds, :], wv_r[:, ds, :])

            # ---- R^T = M^T xq^T :  [d' , q]  (contraction over d_in) ----
            for c in range(2):
                for dso in range(8):
                    ps = psum_mm.tile([P, CHUNK], F32, tag="mm",
                                      name=f"psr_{c}_{dso}")
                    for di in range(8):
                        nc.tensor.matmul(
                            ps[:], m_sb[:, di, dso * P:(dso + 1) * P],
                            xq_t[c][di][:],
                            start=(di == 0), stop=(di == 7))
                    nc.any.tensor_copy(
                        out=rt_sb[:, dso, c * CHUNK:(c + 1) * CHUNK],
                        in_=ps[:])

            # ---------------- attention steps ----------------
            l_ps = [None, None]
            for si, (b, c) in enumerate(STEP_DEF):
                first = FIRST_OF_CHUNK[c] == si
                last = LAST_OF_CHUNK[c] == si
                narrow = si in NARROW_STEPS
                xvt = xv_t[b]
                m_sbt = mask_t[MASKED_STEPS[si]] if si in MASKED_STEPS else None
                if first:
                    l_ps[c] = psum_l.tile([1, CHUNK], F32, tag="l",
                                          name=f"l{c}")
                qcol = slice(c * CHUNK, (c + 1) * CHUNK)
                # S^T tiles + exp + mask + P accumulation
                pts = []
                pacc = pacc_pool.tile([P, CHUNK], BF16, tag="pacc",
                                      name=f"pacc_{si}")
                for ks in range(4):
                    q0 = ks * P if narrow else 0
                    ps_s = psum_mm.tile([P, CHUNK], F32, tag="mm",
                                        name=f"pss_{si}_{ks}")
                    for ds in range(8):
                        nc.tensor.matmul(
                            ps_s[:, q0:],
                            xkt_sb[:, ds, (b * 4 + ks) * P:(b * 4 + ks + 1) * P],
                            rt_sb[:, ds, c * CHUNK + q0:(c + 1) * CHUNK],
                            start=(ds == 0), stop=(ds == 7))
                    pt = pt_pool.tile([P, CHUNK], BF16, tag="pt",
                                      name=f"pt_{si}_{ks}")
                    nc.scalar.activation(pt[:, q0:], ps_s[:, q0:], EXP,
                                         scale=0.03125)
                    if m_sbt is not None:
                        nc.vector.tensor_mul(
                            out=pt[:, q0:], in0=pt[:, q0:],
                            in1=m_sbt[:, ks, q0:])
                    if ks == 0:
                        pass  # pacc starts as pt0 via copy below
                    pts.append(pt)
                nc.vector.tensor_copy(out=pacc[:], in_=pts[0][:])
                for ks in range(1, 4):
                    q0 = ks * P if narrow else 0
                    nc.vector.tensor_add(out=pacc[:, q0:], in0=pacc[:, q0:],
                                         in1=pts[ks][:, q0:])
                # G^T += xv-stationary @ P^T   (per d'-tile)
                for ds in range(8):
                    ps_g = psum_g.tile([P, CHUNK], F32, tag="g",
                                       name=f"psg_{si}_{ds}")
                    for ks in range(4):
                        q0 = ks * P if narrow else 0
                        nc.tensor.matmul(
                            ps_g[:, q0:],
                            xvt[:, ks, ds * P:(ds + 1) * P],
                            pts[ks][:, q0:],
                            start=(ks == 0), stop=(ks == 3))
                    dst = gt_sb[:, ds, qcol]
                    if first:
                        nc.vector.tensor_copy(out=dst, in_=ps_g[:])
                    else:
                        nc.vector.tensor_add(out=dst, in0=dst, in1=ps_g[:])
                    if last:
                        nc.vector.tensor_copy(out=gt16_sb[:, ds, qcol],
                                              in_=dst)
                # l-MM after the G block: its pacc (DVE) dependency is
                # ready by then, so the PE queue never stalls on it.
                nc.tensor.matmul(l_ps[c][:], ones_sb[:], pacc[:],
                                 start=first, stop=last)
                # l finalize: [1,512] -> DRAM bounce -> [128,4] + reciprocal
                if last:
                    l_row = small_pool.tile([1, CHUNK], F32, tag=f"lrow{c}",
                                            name=f"lrow{c}")
                    nc.vector.tensor_copy(out=l_row[:], in_=l_ps[c][:])
                    l_b = dram_pool.tile([CHUNK], F32, tag=f"lb{c}",
                                         name=f"lb{c}")
                    nc.sync.dma_start(
                        l_b[:].rearrange("(a n) -> a n", a=1), l_row[:])
                    l_col = small_pool.tile([P, 4], F32, tag=f"lcol{c}",
                                            name=f"lcol{c}")
                    nc.sync.dma_start(
                        l_col[:], l_b[:].rearrange("(s p) -> p s", p=P))
                    linv = small_pool.tile([P, 4], F32, tag=f"linv{c}",
                                           name=f"linv{c}")
                    nc.vector.reciprocal(linv[:], l_col[:])
                    if c == 0:
                        linv0 = linv
                    else:
                        linv1 = linv

            # ---------------- out = (G Wv) / l ----------------
            for c in range(2):
                linv = linv0 if c == 0 else linv1
                for qs in range(4):
                    for eh in range(2):
                        ps_o = psum_mm.tile([P, CHUNK], F32, tag="mm",
                                            name=f"pso_{c}_{qs}_{eh}")
                        for ds in range(8):
                            nc.tensor.matmul(
                                ps_o[:],
                                gt16_sb[:, ds,
                                        c * CHUNK + qs * P:
                                        c * CHUNK + (qs + 1) * P],
                                wv_sb[:, ds, eh * CHUNK:(eh + 1) * CHUNK],
                                start=(ds == 0), stop=(ds == 7))
                        o_t = o_pool.tile([P, CHUNK], F32, tag="o",
                                          name=f"o_{c}_{qs}_{eh}")
                        nc.scalar.activation(o_t[:], ps_o[:], COPY,
                                             scale=linv[:, qs:qs + 1])
                        r0 = c * CHUNK + qs * P
                        nc.scalar.dma_start(
                            out[r0:r0 + P, eh * CHUNK:(eh + 1) * CHUNK],
                            o_t[:])

    nc.compile()
    return nc


def _get_program():
    if "nc" not in _CACHE:
        _CACHE["nc"] = _build_program()
    return _CACHE["nc"]


def _core_q_rows(core):
    b, half = divmod(core, 2)
    if half == 0:
        lo, hi = 0, 3
    else:
        lo, hi = 1, 2
    return b, lo, hi


def _build_mask(core):
    """masks [4, 128, 4, 512] bf16 for steps si in (0,1,4,5):
    m[mi, p, ks, q] = 1 iff kv_global <= q_global."""
    import ml_dtypes

    _, lo, hi = _core_q_rows(core)
    chunk_start = {0: lo * CHUNK, 1: hi * CHUNK}
    m = np.zeros((4, P, 4, CHUNK), dtype=np.float32)
    kv_local = np.arange(CHUNK)
    q_local = np.arange(CHUNK)
    for si, mi in MASKED_STEPS.items():
        b, c = STEP_DEF[si]
        kv_g = b * CHUNK + kv_local
        q_g = chunk_start[c] + q_local
        allowed = (kv_g[:, None] <= q_g[None, :]).astype(np.float32)
        m[mi] = allowed.reshape(4, P, CHUNK).transpose(1, 0, 2)
    return m.astype(ml_dtypes.bfloat16)


def _make_in_maps(x, wq, wk, wv):
    import ml_dtypes

    m_fold = (wq @ wk.T).astype(ml_dtypes.bfloat16)
    # pack M into 8 contiguous es-column blocks [1024, 128]
    m_packed = np.ascontiguousarray(
        m_fold.reshape(D, 8, P).transpose(1, 0, 2)).reshape(8 * D, P)
    wv16 = wv.astype(ml_dtypes.bfloat16)
    in_maps = []
    for core in range(NCORES):
        b, lo, hi = _core_q_rows(core)
        xb = x[b]
        xbT = np.ascontiguousarray(xb.T).astype(ml_dtypes.bfloat16)
        # pack q columns into 16 contiguous [128, 512] tiles (c*8 + di)
        xq_cols = np.concatenate(
            [xbT[:, lo * CHUNK:(lo + 1) * CHUNK],
             xbT[:, hi * CHUNK:(hi + 1) * CHUNK]], axis=1)  # [1024, 1024]
        xqt = np.ascontiguousarray(
            xq_cols.reshape(8, P, 2, CHUNK).transpose(2, 0, 1, 3)
        ).reshape(16 * P, CHUNK)
        in_maps.append({
            "xqt": xqt,
            "xkt": xbT,
            "xv": np.ascontiguousarray(xb).astype(ml_dtypes.bfloat16),
            "m_in": m_packed,
            "wv": wv16,
            "masks": _build_mask(core),
        })
    return in_maps


def kernel(x, W_query, W_key, W_value):
    from concourse.bass_utils import run_bass_kernel_spmd

    x = np.ascontiguousarray(np.asarray(x, dtype=np.float32))
    wq = np.ascontiguousarray(np.asarray(W_query, dtype=np.float32))
    wk = np.ascontiguousarray(np.asarray(W_key, dtype=np.float32))
    wv = np.ascontiguousarray(np.asarray(W_value, dtype=np.float32))

    nc = _get_program()
    in_maps = _make_in_maps(x, wq, wk, wv)
    res = run_bass_kernel_spmd(nc, in_maps, core_ids=list(range(NCORES)))

    out = np.empty((B, N, D), dtype=np.float32)
    for core in range(NCORES):
        b, lo, hi = _core_q_rows(core)
        o = res.results[core]["out"]
        out[b, lo * CHUNK:(lo + 1) * CHUNK] = o[:CHUNK]
        out[b, hi * CHUNK:(hi + 1) * CHUNK] = o[CHUNK:]
    return out
